# revision 1
# baseline (speedup 1.0000x reference)
"""H2GCN forward on 8 Trainium2 NeuronCores.

out = concat([h0, A1@h0, A2@h0], 1) @ W_out + b_out,  h0 = x @ W1

Data-parallel over destination nodes (1250 rows/core, padded to 1280).
Per core: h0 shard GEMM (fp32r matmuls fed from one blob DMA so each PE
instruction carries at most one sync wait), AllGather of h0 in bf16,
SpMM as dma_gather of source rows + 128-edge selection-matrix bf16
matmuls accumulated in PSUM per dest tile, PE transposes to
feature-major, final fp32r GEMM with the bias applied as a K=1 matmul.
"""
import sys
import types

for _p in ("/opt/trn_rl_repo", "/root/.axon_site", "/root/.axon_site/_ro/trn_rl_repo",
           "/root/.axon_site/_ro/pypackages"):
    if _p not in sys.path:
        sys.path.append(_p)

import numpy as np
import ml_dtypes
import concourse.bass as bass
import concourse.bacc as bacc
import concourse.mybir as mybir
import concourse.tile as tile
from concourse import bass_utils

N, IN_C, HID, OUT_C = 10000, 2048, 256, 256
NCORES = 8
ROWS = N // NCORES          # 1250
PROWS = 1280                # padded (10 x 128)
NT = PROWS // 128
KT = IN_C // 128
ST = NCORES * (PROWS // 128)   # 80 source tiles in the padded AllGather space

f32 = mybir.dt.float32
f32r = mybir.dt.float32r
bf16 = mybir.dt.bfloat16
i16 = mybir.dt.int16
bfnp = ml_dtypes.bfloat16

# blob_a: xT k-tiles then W1 k-tiles (f32 elements per partition row)
OXA, OW1 = 0, KT * PROWS
BLOBA = KT * PROWS + KT * HID
# blob_b: Wout k-tiles, bias (row 0), ones (row 0), identity
OWO, OB, OO, OI = 0, 6 * OUT_C, 6 * OUT_C + OUT_C, 6 * OUT_C + OUT_C + 128
BLOBB = OI + 128

LAST_EXEC_NS = None
LAST_RESULTS = None


def _install_trace_shim():
    try:
        import antenv.axon_hooks  # noqa: F401
        return
    except ImportError:
        pass
    try:
        import antenv
        from trn_agent_boot.trn_boot import _ntff_profile_via_ctypes
        hook = _ntff_profile_via_ctypes("/opt/axon/libaxon_pjrt.so")
        mod = types.ModuleType("antenv.axon_hooks")
        mod.get_axon_ntff_profile_hook = lambda: hook
        mod.set_axon_ntff_profile_hook = lambda h: None
        sys.modules["antenv.axon_hooks"] = mod
        antenv.axon_hooks = mod
    except Exception:
        pass


def _dense_adj(rows, cols, vals, core):
    """Dense padded A^T for this core's dest shard, tiled [128, NT*ST*128]
    bf16 with block (dt, st) at columns (dt*ST+st)*128."""
    lo, hi = core * ROWS, (core + 1) * ROWS
    m = (rows >= lo) & (rows < hi)
    r, c, v = rows[m] - lo, cols[m], vals[m]
    A = np.zeros((NCORES * PROWS, PROWS), np.float32)
    src = (c // ROWS) * PROWS + (c % ROWS)
    np.add.at(A, (src, r), v)
    return np.ascontiguousarray(
        A.reshape(ST, 128, NT, 128).transpose(1, 2, 0, 3)
        .reshape(128, NT * ST * 128)).astype(bfnp)


def _build():
    nc = bacc.Bacc("TRN2", target_bir_lowering=False, debug=False,
                   num_devices=8)
    blob_a = nc.dram_tensor("blob_a", [128, BLOBA], f32r, kind="ExternalInput")
    blob_b = nc.dram_tensor("blob_b", [128, BLOBB], f32r, kind="ExternalInput")
    A1 = nc.dram_tensor("A1", [128, NT * ST * 128], bf16, kind="ExternalInput")
    A2 = nc.dram_tensor("A2", [128, NT * ST * 128], bf16, kind="ExternalInput")
    out = nc.dram_tensor("out", [ROWS, OUT_C], f32, kind="ExternalOutput")

    with tile.TileContext(nc) as tc:
        with tc.tile_pool(name="keep", bufs=1) as keep, \
             tc.tile_pool(name="dram", bufs=1, space="DRAM") as dram, \
             tc.tile_pool(name="pmm", bufs=2, space="PSUM") as pmm, \
             tc.tile_pool(name="psm", bufs=2, space="PSUM") as psm, \
             tc.tile_pool(name="ptr", bufs=2, space="PSUM") as ptr:

            h_sb = keep.tile([128, 3, NT, HID], f32)
            hT = keep.tile([128, 6, PROWS], f32r)
            blob_b_t = keep.tile([128, BLOBB], f32r)
            ident_v = keep.tile([128, 128], f32)
            ag_sb = keep.tile([128, NT, HID], bf16)
            nc.sync.dma_start(blob_b_t[:], blob_b[:])
            # identity produced on DVE so transposes need only one DVE wait
            nc.vector.tensor_copy(ident_v[:], blob_b_t[:, OI:OI + 128].bitcast(f32))

            HT = NT // 2
            ag_in0 = dram.tile([HT * 128, HID], bf16)
            ag_in1 = dram.tile([HT * 128, HID], bf16)
            ag_out0 = dram.tile([NCORES * HT * 128, HID], bf16,
                                addr_space="Shared")
            ag_out1 = dram.tile([NCORES * HT * 128, HID], bf16,
                                addr_space="Shared")

            # ---- phase A: h0 = x @ W1 (local shard), fp32r ----
            with nc.named_scope("h0_gemm"):
                with tc.tile_pool(name="pa", bufs=1) as pa:
                    blob_a_t = pa.tile([128, BLOBA], f32r)
                    qa = BLOBA // 4
                    for q in range(4):
                        nc.sync.dma_start(blob_a_t[:, q * qa:(q + 1) * qa],
                                          blob_a[:, q * qa:(q + 1) * qa])
                    for t in range(NT):
                        ps = pmm.tile([128, HID], f32, tag="mm")
                        for k in range(KT):
                            nc.tensor.matmul(
                                ps[:],
                                blob_a_t[:, OXA + k * PROWS + 128 * t:
                                         OXA + k * PROWS + 128 * (t + 1)],
                                blob_a_t[:, OW1 + k * HID:OW1 + (k + 1) * HID],
                                start=(k == 0), stop=(k == KT - 1),
                            )
                        nc.vector.tensor_copy(h_sb[:, 0, t, :], ps[:])
                        nc.vector.tensor_copy(ag_sb[:, t, :], ps[:])
                        if t == HT - 1:
                            nc.sync.dma_start(
                                ag_in0[:].rearrange("(a p) m -> p a m", p=128),
                                ag_sb[:, 0:HT, :])
                        if t == NT - 1:
                            nc.sync.dma_start(
                                ag_in1[:].rearrange("(a p) m -> p a m", p=128),
                                ag_sb[:, HT:NT, :])

            # ---- phase B: AllGather h0 (bf16), two halves ----
            with nc.named_scope("allgather"):
                nc.gpsimd.collective_compute(
                    "AllGather", mybir.AluOpType.bypass,
                    replica_groups=[list(range(NCORES))],
                    ins=[ag_in0.opt()], outs=[ag_out0.opt()],
                )
                nc.gpsimd.collective_compute(
                    "AllGather", mybir.AluOpType.bypass,
                    replica_groups=[list(range(NCORES))],
                    ins=[ag_in1.opt()], outs=[ag_out1.opt()],
                )

            # ---- phase C: SpMM as dense-block matmuls vs resident h0 ----
            with nc.named_scope("spmm"):
                with tc.tile_pool(name="pc", bufs=1) as pc:
                    h0a = pc.tile([128, ST, HID], bf16)
                    for r in range(NCORES):
                        nc.sync.dma_start(
                            h0a[:, r * NT:r * NT + HT, :],
                            ag_out0[r * HT * 128:(r + 1) * HT * 128, :]
                            .rearrange("(t p) m -> p t m", p=128))
                        nc.sync.dma_start(
                            h0a[:, r * NT + HT:(r + 1) * NT, :],
                            ag_out1[r * HT * 128:(r + 1) * HT * 128, :]
                            .rearrange("(t p) m -> p t m", p=128))
                    st_order = [s for s in range(ST) if s % NT < HT] + \
                               [s for s in range(ST) if s % NT >= HT]
                    for a, A_d in enumerate([A1, A2]):
                        for t in range(NT):
                            a_t = pc.tile([128, ST * 128], bf16, tag="a",
                                          bufs=3)
                            nc.sync.dma_start(
                                a_t[:],
                                A_d[:, t * ST * 128:(t + 1) * ST * 128])
                            ps = psm.tile([128, HID], f32, tag="smm")
                            for i, st in enumerate(st_order):
                                nc.tensor.matmul(
                                    ps[:], a_t[:, 128 * st:128 * (st + 1)],
                                    h0a[:, st, :],
                                    start=(i == 0), stop=(i == ST - 1),
                                )
                            nc.vector.tensor_copy(h_sb[:, 1 + a, t, :], ps[:])

            # ---- phase D: transpose h -> feature-major ----
            with nc.named_scope("transpose"):
                for part in range(3):
                    for t in range(NT):
                        for half in range(2):
                            pst = ptr.tile([128, 128], f32, tag="tr")
                            nc.tensor.transpose(
                                pst[:],
                                h_sb[:, part, t, 128 * half:128 * (half + 1)],
                                ident_v[:],
                            )
                            nc.vector.tensor_copy(
                                hT[:, 2 * part + half, 128 * t:128 * (t + 1)],
                                pst[:])

            # ---- phase E: out = h @ Wout + b (fp32r) ----
            with nc.named_scope("out_gemm"):
                for t in range(NT):
                    ps = pmm.tile([128, OUT_C], f32, tag="mm")
                    nc.tensor.matmul(ps[:], blob_b_t[0:1, OO:OO + 128],
                                     blob_b_t[0:1, OB:OB + OUT_C],
                                     start=True, stop=False)
                    for k in range(6):
                        nc.tensor.matmul(
                            ps[:],
                            hT[:, k, 128 * t:128 * (t + 1)],
                            blob_b_t[:, OWO + k * OUT_C:OWO + (k + 1) * OUT_C],
                            start=False, stop=(k == 5),
                        )
                    o_sb = keep.tile([128, OUT_C], f32, tag="osb", bufs=2)
                    nc.vector.tensor_copy(o_sb[:], ps[:])
                    rows = min(128, ROWS - 128 * t)
                    nc.sync.dma_start(out[128 * t:128 * t + rows, :],
                                      o_sb[:rows, :])
    nc.compile()
    return nc


def kernel(x, adj1_rows, adj1_cols, adj1_vals, adj2_rows, adj2_cols, adj2_vals,
           W1, W_out, b_out):
    global LAST_EXEC_NS, LAST_RESULTS
    _install_trace_shim()
    x = np.asarray(x, np.float32)
    W1 = np.ascontiguousarray(np.asarray(W1, np.float32))
    W_out = np.ascontiguousarray(np.asarray(W_out, np.float32))
    b_out = np.asarray(b_out, np.float32).ravel()

    w1_cols = W1.reshape(KT, 128, HID).transpose(1, 0, 2).reshape(128, KT * HID)
    blob_b = np.zeros((128, BLOBB), np.float32)
    blob_b[:, OWO:OWO + 6 * OUT_C] = \
        W_out.reshape(6, 128, OUT_C).transpose(1, 0, 2).reshape(128, 6 * OUT_C)
    blob_b[0, OB:OB + OUT_C] = b_out
    blob_b[0, OO:OO + 128] = 1.0
    blob_b[:, OI:OI + 128] = np.eye(128, dtype=np.float32)

    in_maps = []
    for c in range(NCORES):
        xtp = np.zeros((IN_C, PROWS), np.float32)
        xtp[:, :ROWS] = x[c * ROWS:(c + 1) * ROWS].T
        blob_a = np.concatenate([
            xtp.reshape(KT, 128, PROWS).transpose(1, 0, 2).reshape(128, KT * PROWS),
            w1_cols,
        ], axis=1)
        in_maps.append({
            "blob_a": blob_a, "blob_b": blob_b,
            "A1": _dense_adj(np.asarray(adj1_rows, np.int64),
                             np.asarray(adj1_cols, np.int64),
                             np.asarray(adj1_vals, np.float32), c),
            "A2": _dense_adj(np.asarray(adj2_rows, np.int64),
                             np.asarray(adj2_cols, np.int64),
                             np.asarray(adj2_vals, np.float32), c),
        })

    nc = _build()
    try:
        res = bass_utils.run_bass_kernel_spmd(
            nc, in_maps, core_ids=list(range(NCORES)), trace=True,
            trace_cores=[0])
    except Exception:
        res = bass_utils.run_bass_kernel_spmd(
            nc, in_maps, core_ids=list(range(NCORES)), trace=False)
    LAST_EXEC_NS = res.exec_time_ns
    LAST_RESULTS = res
    return np.concatenate([res.results[c]["out"] for c in range(NCORES)], axis=0)



# revision 6
# speedup vs baseline: 1.6020x; 1.6020x over previous
"""H2GCN forward on 8 Trainium2 NeuronCores.

out = concat([h0, A1@h0, A2@h0], 1) @ W_out + b_out,  h0 = x @ W1

Data-parallel over destination nodes (1250 rows/core, padded to 1280).
v2 layout:
  - phase A: h0 = x @ W1 in bf16, k-outer loop (10 PSUM accumulators) so
    matmuls start as soon as W1 + first xT k-tile land (no 50us DMA stall).
  - single fp8 AllGather of h0 (halves wire bytes, one RDH floor).
  - SpMM flipped: h1^T/h2^T = (h0^T A) computed with h0 fp8 pairs as the
    stationary operand (DoubleRow: 256 src rows per matmul) and dense fp8
    A^T blocks as the moving operand, accumulated over 40 src-tile pairs
    into 6 PSUM banks (2 feature halves x 3 dst chunks).  Edge values are
    pre-scaled x16/x32 into fp8 range; compensated in W_out rows.
  - h0 transposes (20) run under the AllGather; h1/h2 need no transpose.
  - out = hT @ W_out + b in fp32r as before.
"""
import sys
import types

for _p in ("/opt/trn_rl_repo", "/root/.axon_site", "/root/.axon_site/_ro/trn_rl_repo",
           "/root/.axon_site/_ro/pypackages"):
    if _p not in sys.path:
        sys.path.append(_p)

import numpy as np
import ml_dtypes
import concourse.bass as bass
import concourse.bacc as bacc
import concourse.mybir as mybir
import concourse.tile as tile
from concourse import bass_utils

N, IN_C, HID, OUT_C = 10000, 2048, 256, 256
NCORES = 8
ROWS = N // NCORES          # 1250
PROWS = 1280                # padded (10 x 128)
NT = PROWS // 128           # 10 dst tiles
KT = IN_C // 128            # 16 k tiles
ST = NCORES * NT            # 80 src tiles in padded AllGather space
SP = ST // 2                # 40 src-tile pairs (DoubleRow)
CH = [(0, 512), (512, 512), (1024, 256)]   # dst chunks within 1280

f32 = mybir.dt.float32
f32r = mybir.dt.float32r
bf16 = mybir.dt.bfloat16
f8 = mybir.dt.float8e4
bfnp = ml_dtypes.bfloat16
f8np = ml_dtypes.float8_e4m3

A1_SCALE = 16.0
A2_SCALE = 32.0

# blob_b layout (f32 elems): Wout k-tiles | bias | ones | identity
OWO, OB = 0, 6 * OUT_C
OO, OI = OB + OUT_C, OB + OUT_C + 128
BLOBB = OI + 128

LAST_EXEC_NS = None
LAST_RESULTS = None


def _install_trace_shim():
    try:
        import antenv.axon_hooks  # noqa: F401
        return
    except ImportError:
        pass
    try:
        import antenv
        from trn_agent_boot.trn_boot import _ntff_profile_via_ctypes
        hook = _ntff_profile_via_ctypes("/opt/axon/libaxon_pjrt.so")
        mod = types.ModuleType("antenv.axon_hooks")
        mod.get_axon_ntff_profile_hook = lambda: hook
        mod.set_axon_ntff_profile_hook = lambda h: None
        sys.modules["antenv.axon_hooks"] = mod
        antenv.axon_hooks = mod
    except Exception:
        pass


def _dense_adj(rows, cols, vals, core, scale):
    """Dense padded A^T for this core's dest shard, src-tile-major:
    [128, ST*PROWS] fp8 with src tile s at columns [s*1280, (s+1)*1280)."""
    lo, hi = core * ROWS, (core + 1) * ROWS
    m = (rows >= lo) & (rows < hi)
    r, c, v = rows[m] - lo, cols[m], vals[m] * scale
    A = np.zeros((NCORES * PROWS, PROWS), np.float32)
    src = (c // ROWS) * PROWS + (c % ROWS)
    np.add.at(A, (src, r), v)
    return np.ascontiguousarray(
        A.reshape(ST, 128, PROWS).transpose(1, 0, 2)
        .reshape(128, ST * PROWS)).astype(f8np)


def _build():
    nc = bacc.Bacc("TRN2", target_bir_lowering=False, debug=False,
                   num_devices=8)
    w1_d = nc.dram_tensor("w1", [128, KT * HID], bf16, kind="ExternalInput")
    xt_d = nc.dram_tensor("xt", [128, KT * PROWS], bf16, kind="ExternalInput")
    blob_b = nc.dram_tensor("blob_b", [128, BLOBB], f32r, kind="ExternalInput")
    A1 = nc.dram_tensor("A1", [128, ST * PROWS], f8, kind="ExternalInput")
    A2 = nc.dram_tensor("A2", [128, ST * PROWS], f8, kind="ExternalInput")
    out = nc.dram_tensor("out", [ROWS, OUT_C], f32, kind="ExternalOutput")

    with tile.TileContext(nc) as tc:
        with tc.tile_pool(name="keep", bufs=1) as keep, \
             tc.tile_pool(name="dram", bufs=1, space="DRAM") as dram, \
             tc.tile_pool(name="pT", bufs=1, space="PSUM") as pT:

            h0_sb = keep.tile([128, NT, HID], f32)
            ag_sb = keep.tile([128, NT, HID], f8)
            h0a8 = keep.tile([128, ST, HID], f8)
            hT = keep.tile([128, 6, PROWS], f32r)
            blob_b_t = keep.tile([128, BLOBB], f32r)
            ident_v = keep.tile([128, 128], f32)
            w1_sb = keep.tile([128, KT, HID], bf16)
            nc.sync.dma_start(blob_b_t[:], blob_b[:])
            nc.sync.dma_start(w1_sb[:], w1_d[:].rearrange(
                "p (k m) -> p k m", k=KT))
            # identity produced on DVE so transposes need only one DVE wait
            nc.vector.tensor_copy(ident_v[:], blob_b_t[:, OI:OI + 128].bitcast(f32))

            ag_in = dram.tile([PROWS, HID], f8)
            ag_out = dram.tile([NCORES * PROWS, HID], f8, addr_space="Shared")

            # ---- phase A: h0 = x @ W1 (bf16), k-outer so DMA pipelines.
            # Two halves of 5 PSUM accumulators (PSUM budget); xt k-chunks
            # stay resident so the second half re-reads them from SBUF.
            with nc.named_scope("h0_gemm"):
                with tc.tile_pool(name="pa", bufs=1, space="PSUM") as pa, \
                     tc.tile_pool(name="px", bufs=1) as px:
                    xts = []
                    for k in range(KT):
                        xt_k = px.tile([128, PROWS], bf16, tag=f"xt{k}",
                                       name=f"xt{k}")
                        nc.sync.dma_start(xt_k[:],
                                          xt_d[:, k * PROWS:(k + 1) * PROWS])
                        xts.append(xt_k)
                    for tlo in (0, 5):
                        psA = [pa.tile([128, HID], f32, tag=f"a{i}",
                                       name=f"psA{i}") for i in range(5)]
                        for k in range(KT):
                            for i in range(5):
                                t = tlo + i
                                nc.tensor.matmul(
                                    psA[i][:],
                                    xts[k][:, 128 * t:128 * (t + 1)],
                                    w1_sb[:, k, :],
                                    start=(k == 0), stop=(k == KT - 1),
                                )
                        for i in range(5):
                            t = tlo + i
                            nc.vector.tensor_copy(h0_sb[:, t, :], psA[i][:])
                            nc.vector.tensor_copy(ag_sb[:, t, :], psA[i][:])
                    nc.sync.dma_start(
                        ag_in[:].rearrange("(a p) m -> p a m", p=128),
                        ag_sb[:])

            # ---- phase B: AllGather h0 (fp8), single shot ----
            with nc.named_scope("allgather"):
                nc.gpsimd.collective_compute(
                    "AllGather", mybir.AluOpType.bypass,
                    replica_groups=[list(range(NCORES))],
                    ins=[ag_in.opt()], outs=[ag_out.opt()],
                )

            # ---- phase C: transpose h0 -> feature-major (fills AG window) ----
            with nc.named_scope("transpose"):
                for t in range(NT):
                    for half in range(2):
                        pst = pT.tile([128, 128], f32, tag="tr", bufs=2)
                        nc.tensor.transpose(
                            pst[:],
                            h0_sb[:, t, 128 * half:128 * (half + 1)],
                            ident_v[:],
                        )
                        nc.vector.tensor_copy(
                            hT[:, half, 128 * t:128 * (t + 1)], pst[:])

            # ---- readback: all-gathered h0 (fp8) into SBUF, per-core chunks
            with nc.named_scope("readback"):
                for r in range(NCORES):
                    nc.sync.dma_start(
                        h0a8[:, r * NT:(r + 1) * NT, :],
                        ag_out[r * PROWS:(r + 1) * PROWS, :]
                        .rearrange("(t p) m -> p t m", p=128))

            # ---- phase D: SpMM flipped, fp8 DoubleRow ----
            # hX^T[f, d] = sum_src h0[src, f] * A[src, d]; weights = h0 pairs
            with nc.named_scope("spmm"):
                with tc.tile_pool(name="ps", bufs=1, space="PSUM") as ps, \
                     tc.tile_pool(name="pc", bufs=1) as pc:
                    for a, A_d in enumerate([A1, A2]):
                        psS = {}
                        for fh in range(2):
                            for ci, (co, cw) in enumerate(CH):
                                psS[(fh, ci)] = ps.tile(
                                    [128, cw], f32, tag=f"s{fh}{ci}",
                                    name=f"psS{fh}{ci}")
                        for p in range(SP):
                            a_t = pc.tile([128, 2, PROWS], f8, tag="a",
                                          bufs=10)
                            nc.sync.dma_start(
                                a_t[:],
                                A_d[:, p * 2 * PROWS:(p + 1) * 2 * PROWS]
                                .rearrange("q (two d) -> q two d", two=2))
                            for fh in range(2):
                                for ci, (co, cw) in enumerate(CH):
                                    nc.tensor.matmul(
                                        psS[(fh, ci)][:],
                                        h0a8[:, 2 * p:2 * p + 2,
                                             128 * fh:128 * (fh + 1)],
                                        a_t[:, :, co:co + cw],
                                        start=(p == 0), stop=(p == SP - 1),
                                        perf_mode=mybir.MatmulPerfMode.DoubleRow,
                                    )
                        for fh in range(2):
                            for ci, (co, cw) in enumerate(CH):
                                nc.vector.tensor_copy(
                                    hT[:, 2 + 2 * a + fh, co:co + cw],
                                    psS[(fh, ci)][:])

            # ---- phase E: out = hT @ Wout + b (fp32r) ----
            with nc.named_scope("out_gemm"), \
                 tc.tile_pool(name="po", bufs=1, space="PSUM") as pO:
                for t in range(NT):
                    psO = pO.tile([128, OUT_C], f32, tag="o", bufs=2)
                    nc.tensor.matmul(psO[:], blob_b_t[0:1, OO:OO + 128],
                                     blob_b_t[0:1, OB:OB + OUT_C],
                                     start=True, stop=False)
                    for k in range(6):
                        nc.tensor.matmul(
                            psO[:],
                            hT[:, k, 128 * t:128 * (t + 1)],
                            blob_b_t[:, OWO + k * OUT_C:OWO + (k + 1) * OUT_C],
                            start=False, stop=(k == 5),
                        )
                    o_sb = keep.tile([128, OUT_C], f32, tag="osb", bufs=2)
                    nc.vector.tensor_copy(o_sb[:], psO[:])
                    rows = min(128, ROWS - 128 * t)
                    nc.sync.dma_start(out[128 * t:128 * t + rows, :],
                                      o_sb[:rows, :])
    nc.compile()
    return nc


def kernel(x, adj1_rows, adj1_cols, adj1_vals, adj2_rows, adj2_cols, adj2_vals,
           W1, W_out, b_out):
    global LAST_EXEC_NS, LAST_RESULTS
    _install_trace_shim()
    x = np.asarray(x, np.float32)
    W1 = np.ascontiguousarray(np.asarray(W1, np.float32))
    W_out = np.ascontiguousarray(np.asarray(W_out, np.float32)).copy()
    b_out = np.asarray(b_out, np.float32).ravel()

    # compensate the fp8 edge-value scaling in W_out rows
    W_out[HID:2 * HID] /= A1_SCALE
    W_out[2 * HID:3 * HID] /= A2_SCALE

    w1_b = W1.reshape(KT, 128, HID).transpose(1, 0, 2).reshape(
        128, KT * HID).astype(bfnp)
    blob_b = np.zeros((128, BLOBB), np.float32)
    blob_b[:, OWO:OWO + 6 * OUT_C] = \
        W_out.reshape(6, 128, OUT_C).transpose(1, 0, 2).reshape(128, 6 * OUT_C)
    blob_b[0, OB:OB + OUT_C] = b_out
    blob_b[0, OO:OO + 128] = 1.0
    blob_b[:, OI:OI + 128] = np.eye(128, dtype=np.float32)

    a1r = np.asarray(adj1_rows, np.int64)
    a1c = np.asarray(adj1_cols, np.int64)
    a1v = np.asarray(adj1_vals, np.float32)
    a2r = np.asarray(adj2_rows, np.int64)
    a2c = np.asarray(adj2_cols, np.int64)
    a2v = np.asarray(adj2_vals, np.float32)

    in_maps = []
    for c in range(NCORES):
        xtp = np.zeros((IN_C, PROWS), np.float32)
        xtp[:, :ROWS] = x[c * ROWS:(c + 1) * ROWS].T
        xt_b = xtp.reshape(KT, 128, PROWS).transpose(1, 0, 2).reshape(
            128, KT * PROWS).astype(bfnp)
        in_maps.append({
            "w1": w1_b, "xt": xt_b, "blob_b": blob_b,
            "A1": _dense_adj(a1r, a1c, a1v, c, A1_SCALE),
            "A2": _dense_adj(a2r, a2c, a2v, c, A2_SCALE),
        })

    nc = _build()
    try:
        res = bass_utils.run_bass_kernel_spmd(
            nc, in_maps, core_ids=list(range(NCORES)), trace=True,
            trace_cores=[0])
    except Exception:
        res = bass_utils.run_bass_kernel_spmd(
            nc, in_maps, core_ids=list(range(NCORES)), trace=False)
    LAST_EXEC_NS = res.exec_time_ns
    LAST_RESULTS = res
    return np.concatenate([res.results[c]["out"] for c in range(NCORES)], axis=0)


# revision 8
# speedup vs baseline: 1.6581x; 1.0350x over previous
"""H2GCN forward on 8 Trainium2 NeuronCores.

out = concat([h0, A1@h0, A2@h0], 1) @ W_out + b_out,  h0 = x @ W1

Data-parallel over destination nodes (1250 rows/core, padded to 1280).
v3 layout:
  - phase A: h0 = x @ W1 in bf16, k-outer loop over resident xt chunks,
    tiles 0-5 finished first (6 then 4 PSUM accumulators).
  - AllGather of h0 in fp8, split in two (tiles 0-5, 6-9 of every core)
    so SpMM starts on first-half source pairs while the second half is
    still on the wire.  (The collective subsystem has a ~77us boot
    barrier; both AGs queue right behind it.)
  - SpMM flipped: h1^T/h2^T = h0^T A with h0 fp8 pairs as the stationary
    operand (DoubleRow: 256 src rows per matmul) and dense fp8 A^T blocks
    as the moving operand, accumulated over 40 src-tile pairs into 6 PSUM
    banks (2 feature halves x 3 dst chunks).  Edge values pre-scaled
    x16/x32 into fp8 range; compensated in W_out rows.
  - h0 transposes (20) run under the AG window; h1/h2 need no transpose.
  - out = hT @ W_out + b: h0 contribution in fp32r, h1/h2 in bf16.
"""
import sys
import types

for _p in ("/opt/trn_rl_repo", "/root/.axon_site", "/root/.axon_site/_ro/trn_rl_repo",
           "/root/.axon_site/_ro/pypackages"):
    if _p not in sys.path:
        sys.path.append(_p)

import numpy as np
import ml_dtypes
import concourse.bass as bass
import concourse.bacc as bacc
import concourse.mybir as mybir
import concourse.tile as tile
from concourse import bass_utils

N, IN_C, HID, OUT_C = 10000, 2048, 256, 256
NCORES = 8
ROWS = N // NCORES          # 1250
PROWS = 1280                # padded (10 x 128)
NT = PROWS // 128           # 10 dst tiles
KT = IN_C // 128            # 16 k tiles
ST = NCORES * NT            # 80 src tiles in padded AllGather space
SP = ST // 2                # 40 src-tile pairs (DoubleRow)
CH = [(0, 512), (512, 512), (1024, 256)]   # dst chunks within 1280
HA, HB = 6, 4               # AllGather half sizes (tiles per core)

f32 = mybir.dt.float32
f32r = mybir.dt.float32r
bf16 = mybir.dt.bfloat16
f8 = mybir.dt.float8e4
bfnp = ml_dtypes.bfloat16
f8np = ml_dtypes.float8_e4m3

A1_SCALE = 16.0
A2_SCALE = 32.0

# blob_b layout (f32 elems): Wout k-tiles 0-1 | bias | ones | identity
OWO, OB = 0, 2 * OUT_C
OO, OI = OB + OUT_C, OB + OUT_C + 128
BLOBB = OI + 128

# SpMM pair order: first-half pairs (j 0-2 of each core), then second half
PAIR_ORDER = [r * 5 + j for r in range(NCORES) for j in range(3)] + \
             [r * 5 + j for r in range(NCORES) for j in (3, 4)]

LAST_EXEC_NS = None
LAST_RESULTS = None


def _install_trace_shim():
    try:
        import antenv.axon_hooks  # noqa: F401
        return
    except ImportError:
        pass
    try:
        import antenv
        from trn_agent_boot.trn_boot import _ntff_profile_via_ctypes
        hook = _ntff_profile_via_ctypes("/opt/axon/libaxon_pjrt.so")
        mod = types.ModuleType("antenv.axon_hooks")
        mod.get_axon_ntff_profile_hook = lambda: hook
        mod.set_axon_ntff_profile_hook = lambda h: None
        sys.modules["antenv.axon_hooks"] = mod
        antenv.axon_hooks = mod
    except Exception:
        pass


def _dense_adj(rows, cols, vals, core, scale):
    """Dense padded A^T for this core's dest shard, src-tile-major:
    [128, ST*PROWS] fp8 with src tile s at columns [s*1280, (s+1)*1280)."""
    lo, hi = core * ROWS, (core + 1) * ROWS
    m = (rows >= lo) & (rows < hi)
    r, c, v = rows[m] - lo, cols[m], vals[m] * scale
    A = np.zeros((NCORES * PROWS, PROWS), np.float32)
    src = (c // ROWS) * PROWS + (c % ROWS)
    np.add.at(A, (src, r), v)
    return np.ascontiguousarray(
        A.reshape(ST, 128, PROWS).transpose(1, 0, 2)
        .reshape(128, ST * PROWS)).astype(f8np)


def _build():
    nc = bacc.Bacc("TRN2", target_bir_lowering=False, debug=False,
                   num_devices=8)
    w1_d = nc.dram_tensor("w1", [128, KT * HID], bf16, kind="ExternalInput")
    xt_d = nc.dram_tensor("xt", [128, KT * PROWS], bf16, kind="ExternalInput")
    blob_b = nc.dram_tensor("blob_b", [128, BLOBB], f32r, kind="ExternalInput")
    wo8_d = nc.dram_tensor("wo8", [128, 4 * OUT_C], bf16, kind="ExternalInput")
    A1 = nc.dram_tensor("A1", [128, ST * PROWS], f8, kind="ExternalInput")
    A2 = nc.dram_tensor("A2", [128, ST * PROWS], f8, kind="ExternalInput")
    out = nc.dram_tensor("out", [ROWS, OUT_C], f32, kind="ExternalOutput")

    with tile.TileContext(nc) as tc:
        with tc.tile_pool(name="keep", bufs=1) as keep, \
             tc.tile_pool(name="dram", bufs=1, space="DRAM") as dram, \
             tc.tile_pool(name="pT", bufs=1, space="PSUM") as pT:

            h0_sb = keep.tile([128, NT, HID], f32)
            ag_sb = keep.tile([128, NT, HID], f8)
            h0a8 = keep.tile([128, ST, HID], f8)
            hT = keep.tile([128, 2, PROWS], f32r)
            hT8 = keep.tile([128, 4, PROWS], bf16)
            blob_b_t = keep.tile([128, BLOBB], f32r)
            wo8_sb = keep.tile([128, 4, OUT_C], bf16)
            ident_v = keep.tile([128, 128], f32)
            w1_sb = keep.tile([128, KT, HID], bf16)
            nc.sync.dma_start(w1_sb[:], w1_d[:].rearrange(
                "p (k m) -> p k m", k=KT))
            nc.sync.dma_start(blob_b_t[:], blob_b[:])
            nc.sync.dma_start(wo8_sb[:], wo8_d[:].rearrange(
                "p (k m) -> p k m", k=4))
            # identity produced on DVE so transposes need only one DVE wait
            nc.vector.tensor_copy(ident_v[:], blob_b_t[:, OI:OI + 128].bitcast(f32))

            ag_in0 = dram.tile([HA * 128, HID], f8)
            ag_in1 = dram.tile([HB * 128, HID], f8)
            ag_out0 = dram.tile([NCORES * HA * 128, HID], f8,
                                addr_space="Shared")
            ag_out1 = dram.tile([NCORES * HB * 128, HID], f8,
                                addr_space="Shared")

            # ---- phase A: h0 = x @ W1 (bf16), k-outer so DMA pipelines.
            # Tiles 0-5 first so the first AllGather half can launch early.
            with nc.named_scope("h0_gemm"):
                with tc.tile_pool(name="pa", bufs=1, space="PSUM") as pa, \
                     tc.tile_pool(name="px", bufs=1) as px:
                    xts = []
                    for k in range(KT):
                        xt_k = px.tile([128, PROWS], bf16, tag=f"xt{k}",
                                       name=f"xt{k}")
                        nc.sync.dma_start(xt_k[:],
                                          xt_d[:, k * PROWS:(k + 1) * PROWS])
                        xts.append(xt_k)
                    for tlo, tn in ((0, HA), (HA, HB)):
                        psA = [pa.tile([128, HID], f32, tag=f"a{i}",
                                       name=f"psA{i}") for i in range(tn)]
                        for k in range(KT):
                            for i in range(tn):
                                t = tlo + i
                                nc.tensor.matmul(
                                    psA[i][:],
                                    xts[k][:, 128 * t:128 * (t + 1)],
                                    w1_sb[:, k, :],
                                    start=(k == 0), stop=(k == KT - 1),
                                )
                        for i in range(tn):
                            t = tlo + i
                            nc.vector.tensor_copy(h0_sb[:, t, :], psA[i][:])
                            nc.vector.tensor_copy(ag_sb[:, t, :], psA[i][:])
                        if tlo == 0:
                            nc.sync.dma_start(
                                ag_in0[:].rearrange("(a p) m -> p a m", p=128),
                                ag_sb[:, 0:HA, :])
                        else:
                            nc.sync.dma_start(
                                ag_in1[:].rearrange("(a p) m -> p a m", p=128),
                                ag_sb[:, HA:NT, :])

            # ---- phase B: AllGather h0 (fp8), two halves ----
            with nc.named_scope("allgather"):
                nc.gpsimd.collective_compute(
                    "AllGather", mybir.AluOpType.bypass,
                    replica_groups=[list(range(NCORES))],
                    ins=[ag_in0.opt()], outs=[ag_out0.opt()],
                )
                nc.gpsimd.collective_compute(
                    "AllGather", mybir.AluOpType.bypass,
                    replica_groups=[list(range(NCORES))],
                    ins=[ag_in1.opt()], outs=[ag_out1.opt()],
                )

            # ---- phase C: transpose h0 -> feature-major (fills AG window) ----
            with nc.named_scope("transpose"):
                for t in range(NT):
                    for half in range(2):
                        pst = pT.tile([128, 128], f32, tag="tr", bufs=2)
                        nc.tensor.transpose(
                            pst[:],
                            h0_sb[:, t, 128 * half:128 * (half + 1)],
                            ident_v[:],
                        )
                        nc.vector.tensor_copy(
                            hT[:, half, 128 * t:128 * (t + 1)], pst[:])

            # ---- readback: all-gathered h0 (fp8) into SBUF, per-core chunks
            with nc.named_scope("readback"):
                for r in range(NCORES):
                    nc.sync.dma_start(
                        h0a8[:, r * NT:r * NT + HA, :],
                        ag_out0[r * HA * 128:(r + 1) * HA * 128, :]
                        .rearrange("(t p) m -> p t m", p=128))
                for r in range(NCORES):
                    nc.sync.dma_start(
                        h0a8[:, r * NT + HA:(r + 1) * NT, :],
                        ag_out1[r * HB * 128:(r + 1) * HB * 128, :]
                        .rearrange("(t p) m -> p t m", p=128))

            # ---- phase D: SpMM flipped, fp8 DoubleRow ----
            # hX^T[f, d] = sum_src h0[src, f] * A[src, d]; weights = h0 pairs
            with nc.named_scope("spmm"):
                with tc.tile_pool(name="ps", bufs=1, space="PSUM") as ps, \
                     tc.tile_pool(name="pc", bufs=1) as pc:
                    for a, A_d in enumerate([A1, A2]):
                        psS = {}
                        for fh in range(2):
                            for ci, (co, cw) in enumerate(CH):
                                psS[(fh, ci)] = ps.tile(
                                    [128, cw], f32, tag=f"s{fh}{ci}",
                                    name=f"psS{fh}{ci}")
                        for pi, p in enumerate(PAIR_ORDER):
                            a_t = pc.tile([128, 2, PROWS], f8, tag="a",
                                          bufs=10)
                            nc.sync.dma_start(
                                a_t[:],
                                A_d[:, p * 2 * PROWS:(p + 1) * 2 * PROWS]
                                .rearrange("q (two d) -> q two d", two=2))
                            for fh in range(2):
                                for ci, (co, cw) in enumerate(CH):
                                    nc.tensor.matmul(
                                        psS[(fh, ci)][:],
                                        h0a8[:, 2 * p:2 * p + 2,
                                             128 * fh:128 * (fh + 1)],
                                        a_t[:, :, co:co + cw],
                                        start=(pi == 0), stop=(pi == SP - 1),
                                        perf_mode=mybir.MatmulPerfMode.DoubleRow,
                                    )
                        for fh in range(2):
                            for ci, (co, cw) in enumerate(CH):
                                nc.vector.tensor_copy(
                                    hT8[:, 2 * a + fh, co:co + cw],
                                    psS[(fh, ci)][:])

            # ---- phase E: out = hT @ Wout + b (h0 fp32r, h1/h2 bf16) ----
            with nc.named_scope("out_gemm"), \
                 tc.tile_pool(name="po", bufs=1, space="PSUM") as pO:
                for t in range(NT):
                    psO = pO.tile([128, OUT_C], f32, tag="o", bufs=2)
                    nc.tensor.matmul(psO[:], blob_b_t[0:1, OO:OO + 128],
                                     blob_b_t[0:1, OB:OB + OUT_C],
                                     start=True, stop=False)
                    for k in range(2):
                        nc.tensor.matmul(
                            psO[:],
                            hT[:, k, 128 * t:128 * (t + 1)],
                            blob_b_t[:, OWO + k * OUT_C:OWO + (k + 1) * OUT_C],
                            start=False, stop=False,
                        )
                    for k in range(4):
                        nc.tensor.matmul(
                            psO[:],
                            hT8[:, k, 128 * t:128 * (t + 1)],
                            wo8_sb[:, k, :],
                            start=False, stop=(k == 3),
                        )
                    o_sb = keep.tile([128, OUT_C], f32, tag="osb", bufs=2)
                    nc.vector.tensor_copy(o_sb[:], psO[:])
                    rows = min(128, ROWS - 128 * t)
                    nc.sync.dma_start(out[128 * t:128 * t + rows, :],
                                      o_sb[:rows, :])
    nc.compile()
    return nc


def kernel(x, adj1_rows, adj1_cols, adj1_vals, adj2_rows, adj2_cols, adj2_vals,
           W1, W_out, b_out):
    global LAST_EXEC_NS, LAST_RESULTS
    _install_trace_shim()
    x = np.asarray(x, np.float32)
    W1 = np.ascontiguousarray(np.asarray(W1, np.float32))
    W_out = np.ascontiguousarray(np.asarray(W_out, np.float32)).copy()
    b_out = np.asarray(b_out, np.float32).ravel()

    # compensate the fp8 edge-value scaling in W_out rows
    W_out[HID:2 * HID] /= A1_SCALE
    W_out[2 * HID:3 * HID] /= A2_SCALE

    w1_b = W1.reshape(KT, 128, HID).transpose(1, 0, 2).reshape(
        128, KT * HID).astype(bfnp)
    blob_b = np.zeros((128, BLOBB), np.float32)
    blob_b[:, OWO:OWO + 2 * OUT_C] = \
        W_out[:2 * 128].reshape(2, 128, OUT_C).transpose(1, 0, 2).reshape(
            128, 2 * OUT_C)
    blob_b[0, OB:OB + OUT_C] = b_out
    blob_b[0, OO:OO + 128] = 1.0
    blob_b[:, OI:OI + 128] = np.eye(128, dtype=np.float32)
    wo8 = W_out[2 * 128:].reshape(4, 128, OUT_C).transpose(1, 0, 2).reshape(
        128, 4 * OUT_C).astype(bfnp)

    a1r = np.asarray(adj1_rows, np.int64)
    a1c = np.asarray(adj1_cols, np.int64)
    a1v = np.asarray(adj1_vals, np.float32)
    a2r = np.asarray(adj2_rows, np.int64)
    a2c = np.asarray(adj2_cols, np.int64)
    a2v = np.asarray(adj2_vals, np.float32)

    in_maps = []
    for c in range(NCORES):
        xtp = np.zeros((IN_C, PROWS), np.float32)
        xtp[:, :ROWS] = x[c * ROWS:(c + 1) * ROWS].T
        xt_b = xtp.reshape(KT, 128, PROWS).transpose(1, 0, 2).reshape(
            128, KT * PROWS).astype(bfnp)
        in_maps.append({
            "w1": w1_b, "xt": xt_b, "blob_b": blob_b, "wo8": wo8,
            "A1": _dense_adj(a1r, a1c, a1v, c, A1_SCALE),
            "A2": _dense_adj(a2r, a2c, a2v, c, A2_SCALE),
        })

    nc = _build()
    try:
        res = bass_utils.run_bass_kernel_spmd(
            nc, in_maps, core_ids=list(range(NCORES)), trace=True,
            trace_cores=[0])
    except Exception:
        res = bass_utils.run_bass_kernel_spmd(
            nc, in_maps, core_ids=list(range(NCORES)), trace=False)
    LAST_EXEC_NS = res.exec_time_ns
    LAST_RESULTS = res
    return np.concatenate([res.results[c]["out"] for c in range(NCORES)], axis=0)


# revision 19
# speedup vs baseline: 1.7243x; 1.0399x over previous
"""H2GCN forward on 8 Trainium2 NeuronCores.

out = concat([h0, A1@h0, A2@h0], 1) @ W_out + b_out,  h0 = x @ W1

Data-parallel over destination nodes (1250 rows/core, padded to 1280).
v3 layout:
  - phase A: h0 = x @ W1 in bf16, k-outer loop over resident xt chunks,
    tiles 0-5 finished first (6 then 4 PSUM accumulators).
  - AllGather of h0 in fp8, split in two (tiles 0-5, 6-9 of every core)
    so SpMM starts on first-half source pairs while the second half is
    still on the wire.  (The collective subsystem has a ~77us boot
    barrier; both AGs queue right behind it.)
  - SpMM flipped: h1^T/h2^T = h0^T A with h0 fp8 pairs as the stationary
    operand (DoubleRow: 256 src rows per matmul) and dense fp8 A^T blocks
    as the moving operand, accumulated over 40 src-tile pairs into 6 PSUM
    banks (2 feature halves x 3 dst chunks).  Edge values pre-scaled
    x16/x32 into fp8 range; compensated in W_out rows.
  - h0 transposes (20) run under the AG window; h1/h2 need no transpose.
  - out = hT @ W_out + b: h0 contribution in fp32r, h1/h2 in bf16.
"""
import sys
import types

for _p in ("/opt/trn_rl_repo", "/root/.axon_site", "/root/.axon_site/_ro/trn_rl_repo",
           "/root/.axon_site/_ro/pypackages"):
    if _p not in sys.path:
        sys.path.append(_p)

import numpy as np
import ml_dtypes
import concourse.bass as bass
import concourse.bacc as bacc
import concourse.mybir as mybir
import concourse.tile as tile
from concourse import bass_utils

N, IN_C, HID, OUT_C = 10000, 2048, 256, 256
NCORES = 8
ROWS = N // NCORES          # 1250
PROWS = 1280                # padded (10 x 128)
NT = PROWS // 128           # 10 dst tiles
KT = IN_C // 128            # 16 k tiles
ST = NCORES * NT            # 80 src tiles in padded AllGather space
SP = ST // 2                # 40 src-tile pairs (DoubleRow)
CH = [(0, 512), (512, 512), (1024, 256)]   # dst chunks within 1280
HA, HB = 8, 2               # AllGather half sizes (tiles per core)

f32 = mybir.dt.float32
f32r = mybir.dt.float32r
bf16 = mybir.dt.bfloat16
f8 = mybir.dt.float8e4
bfnp = ml_dtypes.bfloat16
f8np = ml_dtypes.float8_e4m3

A1_SCALE = 16.0
A2_SCALE = 32.0

# wo8 layout (bf16 elems): Wout k-tiles 0-5 | bias (row 0) | ones (row 0)
WO8, OB8 = 0, 6 * OUT_C
OO8 = OB8 + OUT_C
WO8N = OO8 + 128

# SpMM pair order: first-half pairs (j 0-3 of each core), then second half
PAIR_ORDER = [r * 5 + j for r in range(NCORES) for j in range(4)] + \
             [r * 5 + 4 for r in range(NCORES)]

LAST_EXEC_NS = None
LAST_RESULTS = None


def _install_trace_shim():
    try:
        import antenv.axon_hooks  # noqa: F401
        return
    except ImportError:
        pass
    try:
        import antenv
        from trn_agent_boot.trn_boot import _ntff_profile_via_ctypes
        hook = _ntff_profile_via_ctypes("/opt/axon/libaxon_pjrt.so")
        mod = types.ModuleType("antenv.axon_hooks")
        mod.get_axon_ntff_profile_hook = lambda: hook
        mod.set_axon_ntff_profile_hook = lambda h: None
        sys.modules["antenv.axon_hooks"] = mod
        antenv.axon_hooks = mod
    except Exception:
        pass


def _dense_adj(rows, cols, vals, core, scale):
    """Dense padded A^T for this core's dest shard, src-tile-major:
    [128, ST*PROWS] fp8 with src tile s at columns [s*1280, (s+1)*1280)."""
    lo, hi = core * ROWS, (core + 1) * ROWS
    m = (rows >= lo) & (rows < hi)
    r, c, v = rows[m] - lo, cols[m], vals[m] * scale
    A = np.zeros((NCORES * PROWS, PROWS), np.float32)
    src = (c // ROWS) * PROWS + (c % ROWS)
    np.add.at(A, (src, r), v)
    return np.ascontiguousarray(
        A.reshape(ST, 128, PROWS).transpose(1, 0, 2)
        .reshape(128, ST * PROWS)).astype(f8np)


def _build():
    nc = bacc.Bacc("TRN2", target_bir_lowering=False, debug=False,
                   num_devices=8)
    w1_d = nc.dram_tensor("w1", [128, KT * HID], bf16, kind="ExternalInput")
    xt_d = nc.dram_tensor("xt", [128, KT * PROWS], bf16, kind="ExternalInput")
    ident_d = nc.dram_tensor("ident", [128, 128], f32, kind="ExternalInput")
    wo8_d = nc.dram_tensor("wo8", [128, WO8N], bf16, kind="ExternalInput")
    A1 = nc.dram_tensor("A1", [128, ST * PROWS], f8, kind="ExternalInput")
    A2 = nc.dram_tensor("A2", [128, ST * PROWS], f8, kind="ExternalInput")
    out = nc.dram_tensor("out", [ROWS, OUT_C], f32, kind="ExternalOutput")

    with tile.TileContext(nc) as tc:
        with tc.tile_pool(name="keep", bufs=1) as keep, \
             tc.tile_pool(name="dram", bufs=1, space="DRAM") as dram, \
             tc.tile_pool(name="pT", bufs=1, space="PSUM") as pT:

            h0_sb = keep.tile([128, NT, HID], f32)
            ag_sb = keep.tile([128, NT, HID], f8)
            h0a8 = keep.tile([128, ST, HID], f8)
            hT8 = keep.tile([128, 6, PROWS], bf16)
            wo8_sb = keep.tile([128, WO8N], bf16)
            ident_v = keep.tile([128, 128], f32)
            w1_sb = keep.tile([128, KT, HID], bf16)
            nc.sync.dma_start(w1_sb[:], w1_d[:].rearrange(
                "p (k m) -> p k m", k=KT))
            nc.sync.dma_start(wo8_sb[:], wo8_d[:])
            ident_t = keep.tile([128, 128], f32)
            nc.sync.dma_start(ident_t[:], ident_d[:])
            # identity produced on DVE so transposes need only one DVE wait
            nc.vector.tensor_copy(ident_v[:], ident_t[:])

            ag_in0 = dram.tile([HA * 128, HID], f8)
            ag_in1 = dram.tile([HB * 128, HID], f8)
            ag_out0 = dram.tile([NCORES * HA * 128, HID], f8,
                                addr_space="Shared")
            ag_out1 = dram.tile([NCORES * HB * 128, HID], f8,
                                addr_space="Shared")

            # ---- phase A: h0 = x @ W1 (bf16), k-outer so DMA pipelines.
            # Tiles 0-5 first so the first AllGather half can launch early.
            with nc.named_scope("h0_gemm"):
                with tc.tile_pool(name="pa", bufs=1, space="PSUM") as pa, \
                     tc.tile_pool(name="px", bufs=1) as px:
                    xts = []
                    for k in range(KT):
                        xt_k = px.tile([128, PROWS], bf16, tag=f"xt{k}",
                                       name=f"xt{k}")
                        nc.sync.dma_start(xt_k[:],
                                          xt_d[:, k * PROWS:(k + 1) * PROWS])
                        xts.append(xt_k)
                    for tlo, tn in ((0, 5), (5, 5)):
                        psA = [pa.tile([128, HID], f32, tag=f"a{i}",
                                       name=f"psA{i}") for i in range(tn)]
                        for k in range(KT):
                            for i in range(tn):
                                t = tlo + i
                                nc.tensor.matmul(
                                    psA[i][:],
                                    xts[k][:, 128 * t:128 * (t + 1)],
                                    w1_sb[:, k, :],
                                    start=(k == 0), stop=(k == KT - 1),
                                )
                        for i in range(tn):
                            t = tlo + i
                            nc.vector.tensor_copy(h0_sb[:, t, :], psA[i][:])
                            nc.vector.tensor_copy(ag_sb[:, t, :], psA[i][:])
                    nc.sync.dma_start(
                        ag_in0[:].rearrange("(a p) m -> p a m", p=128),
                        ag_sb[:, 0:HA, :])
                    nc.sync.dma_start(
                        ag_in1[:].rearrange("(a p) m -> p a m", p=128),
                        ag_sb[:, HA:NT, :])

            # ---- phase B: AllGather h0 (fp8), two halves ----
            with nc.named_scope("allgather"):
                nc.gpsimd.collective_compute(
                    "AllGather", mybir.AluOpType.bypass,
                    replica_groups=[list(range(NCORES))],
                    ins=[ag_in0.opt()], outs=[ag_out0.opt()],
                )
                nc.gpsimd.collective_compute(
                    "AllGather", mybir.AluOpType.bypass,
                    replica_groups=[list(range(NCORES))],
                    ins=[ag_in1.opt()], outs=[ag_out1.opt()],
                )

            # ---- phase C: transpose h0 -> feature-major (fills AG window) ----
            with nc.named_scope("transpose"):
                for t in range(NT):
                    for half in range(2):
                        pst = pT.tile([128, 128], f32, tag="tr", bufs=2)
                        nc.tensor.transpose(
                            pst[:],
                            h0_sb[:, t, 128 * half:128 * (half + 1)],
                            ident_v[:],
                        )
                        nc.vector.tensor_copy(
                            hT8[:, half, 128 * t:128 * (t + 1)], pst[:])

            # ---- readback: all-gathered h0 (fp8) into SBUF, per-core chunks
            with nc.named_scope("readback"):
                for r in range(NCORES):
                    nc.sync.dma_start(
                        h0a8[:, r * NT:r * NT + HA, :],
                        ag_out0[r * HA * 128:(r + 1) * HA * 128, :]
                        .rearrange("(t p) m -> p t m", p=128))
                for r in range(NCORES):
                    nc.sync.dma_start(
                        h0a8[:, r * NT + HA:(r + 1) * NT, :],
                        ag_out1[r * HB * 128:(r + 1) * HB * 128, :]
                        .rearrange("(t p) m -> p t m", p=128))

            # ---- phase D: SpMM flipped, fp8 DoubleRow ----
            # hX^T[f, d] = sum_src h0[src, f] * A[src, d]; weights = h0 pairs
            with nc.named_scope("spmm"):
                with tc.tile_pool(name="ps", bufs=1, space="PSUM") as ps, \
                     tc.tile_pool(name="pc", bufs=1) as pc:
                    for a, A_d in enumerate([A1, A2]):
                        psS = {}
                        for fh in range(2):
                            for ci, (co, cw) in enumerate(CH):
                                psS[(fh, ci)] = ps.tile(
                                    [128, cw], f32, tag=f"s{fh}{ci}",
                                    name=f"psS{fh}{ci}")
                        for pi, p in enumerate(PAIR_ORDER):
                            a_t = pc.tile([128, 2, PROWS], f8, tag="a",
                                          bufs=24)
                            nc.sync.dma_start(
                                a_t[:],
                                A_d[:, p * 2 * PROWS:(p + 1) * 2 * PROWS]
                                .rearrange("q (two d) -> q two d", two=2))
                            for fh in range(2):
                                for ci, (co, cw) in enumerate(CH):
                                    nc.tensor.matmul(
                                        psS[(fh, ci)][:],
                                        h0a8[:, 2 * p:2 * p + 2,
                                             128 * fh:128 * (fh + 1)],
                                        a_t[:, :, co:co + cw],
                                        start=(pi == 0), stop=(pi == SP - 1),
                                        perf_mode=mybir.MatmulPerfMode.DoubleRow,
                                    )
                        for fh in range(2):
                            for ci, (co, cw) in enumerate(CH):
                                nc.vector.tensor_copy(
                                    hT8[:, 2 + 2 * a + fh, co:co + cw],
                                    psS[(fh, ci)][:])

            # ---- phase E: out = hT @ Wout + b (h0 fp32r, h1/h2 bf16) ----
            with nc.named_scope("out_gemm"), \
                 tc.tile_pool(name="po", bufs=1, space="PSUM") as pO:
                for t in range(NT):
                    psO = pO.tile([128, OUT_C], f32, tag="o", bufs=2)
                    nc.tensor.matmul(psO[:], wo8_sb[0:1, OO8:OO8 + 128],
                                     wo8_sb[0:1, OB8:OB8 + OUT_C],
                                     start=True, stop=False)
                    for k in range(6):
                        nc.tensor.matmul(
                            psO[:],
                            hT8[:, k, 128 * t:128 * (t + 1)],
                            wo8_sb[:, WO8 + k * OUT_C:WO8 + (k + 1) * OUT_C],
                            start=False, stop=(k == 5),
                        )
                    o_sb = keep.tile([128, OUT_C], f32, tag="osb", bufs=2)
                    nc.vector.tensor_copy(o_sb[:], psO[:])
                    rows = min(128, ROWS - 128 * t)
                    nc.sync.dma_start(out[128 * t:128 * t + rows, :],
                                      o_sb[:rows, :])
    nc.compile()
    return nc


def kernel(x, adj1_rows, adj1_cols, adj1_vals, adj2_rows, adj2_cols, adj2_vals,
           W1, W_out, b_out):
    global LAST_EXEC_NS, LAST_RESULTS
    _install_trace_shim()
    x = np.asarray(x, np.float32)
    W1 = np.ascontiguousarray(np.asarray(W1, np.float32))
    W_out = np.ascontiguousarray(np.asarray(W_out, np.float32)).copy()
    b_out = np.asarray(b_out, np.float32).ravel()

    # compensate the fp8 edge-value scaling in W_out rows
    W_out[HID:2 * HID] /= A1_SCALE
    W_out[2 * HID:3 * HID] /= A2_SCALE

    w1_b = W1.reshape(KT, 128, HID).transpose(1, 0, 2).reshape(
        128, KT * HID).astype(bfnp)
    wo8 = np.zeros((128, WO8N), np.float32)
    wo8[:, WO8:WO8 + 6 * OUT_C] = \
        W_out.reshape(6, 128, OUT_C).transpose(1, 0, 2).reshape(128, 6 * OUT_C)
    wo8[0, OB8:OB8 + OUT_C] = b_out
    wo8[0, OO8:OO8 + 128] = 1.0
    wo8 = wo8.astype(bfnp)
    ident = np.eye(128, dtype=np.float32)

    a1r = np.asarray(adj1_rows, np.int64)
    a1c = np.asarray(adj1_cols, np.int64)
    a1v = np.asarray(adj1_vals, np.float32)
    a2r = np.asarray(adj2_rows, np.int64)
    a2c = np.asarray(adj2_cols, np.int64)
    a2v = np.asarray(adj2_vals, np.float32)

    in_maps = []
    for c in range(NCORES):
        xtp = np.zeros((IN_C, PROWS), np.float32)
        xtp[:, :ROWS] = x[c * ROWS:(c + 1) * ROWS].T
        xt_b = xtp.reshape(KT, 128, PROWS).transpose(1, 0, 2).reshape(
            128, KT * PROWS).astype(bfnp)
        in_maps.append({
            "w1": w1_b, "xt": xt_b, "ident": ident, "wo8": wo8,
            "A1": _dense_adj(a1r, a1c, a1v, c, A1_SCALE),
            "A2": _dense_adj(a2r, a2c, a2v, c, A2_SCALE),
        })

    nc = _build()
    try:
        res = bass_utils.run_bass_kernel_spmd(
            nc, in_maps, core_ids=list(range(NCORES)), trace=True,
            trace_cores=[0])
    except Exception:
        res = bass_utils.run_bass_kernel_spmd(
            nc, in_maps, core_ids=list(range(NCORES)), trace=False)
    LAST_EXEC_NS = res.exec_time_ns
    LAST_RESULTS = res
    return np.concatenate([res.results[c]["out"] for c in range(NCORES)], axis=0)
